# revision 1
# baseline (speedup 1.0000x reference)
"""Trainium2 Bass kernel for nn_CNLinkPredictor (gnn_message_passing).

Strategy: data-parallel over target edges T (8192) across 8 NeuronCores
(1024 edges/core). Per core, the per-edge CN-token transformer is computed
in tiles of 4 edges = 128 tokens (tokens on SBUF partitions).

Layout/algorithm notes:
 - pf = [xw|xi|xj|xi*xj] @ tok_w.T is split: the xw part is a per-token
   matmul; the (xi,xj,xi*xj) part depends only on the edge -> computed once
   per edge ("EC") and broadcast to the edge's 32 tokens via a rank-4 matmul.
 - LayerNorm affine transforms (gamma/beta) are folded into the following
   matmul weights on the host; device LN computes only (x-mu)*rsqrt(var+eps).
 - qT/kT are produced channel-major with heads padded to 32-partition slots
   so score matmuls can use 32-aligned lhsT partition slices.
 - scores land in PSUM [128 ktok, 8 heads x 128 qtok]; masking (block-diag
   cross-edge + key-padding) is folded into the exp's per-partition bias
   (4 activation calls, one per edge in the tile).
 - softmax denominators ride along as a 17th column of V ("aug-V"); ctx
   matmuls produce [17, 128] blocks (16 ctx channels + denominator row) in
   32-partition head slots; normalization = stream_shuffle + divide.
 - v bias, out_proj bias, ff biases etc. are folded on host where linear.
"""

import sys
import threading

sys.path.insert(0, "/opt/trn_rl_repo")

import numpy as np

import concourse.bass as bass
import concourse.bacc as bacc
import concourse.mybir as mybir
from concourse.tile import TileContext
from concourse.masks import make_identity
from concourse.bass_utils import run_bass_kernel_spmd

F32 = mybir.dt.float32
I32 = mybir.dt.int32
AF = mybir.ActivationFunctionType
ALU = mybir.AluOpType

N, C, H, O = 100000, 128, 256, 1
T, K = 8192, 32
NHEAD, DH, FF = 8, 16, 512
NCORES = 8
TC = T // NCORES          # 1024 edges per core
NT = TC * K // 128        # 256 main tiles (4 edges / 128 tokens each)
NE = TC // 128            # 8 edge tiles (phase A)
NEG = -1e9
EPS_DENOM = 1e-30


def _build_nc(nt=NT, phases="abc", bsteps=99):
    assert nt % 32 == 0
    tcn = 4 * nt           # edges covered by this build
    ne = tcn // 128        # phase-A tiles
    nc = bacc.Bacc("TRN2", target_bir_lowering=False)

    dt = {}

    def din(name, shape, dtype=F32):
        dt[name] = nc.dram_tensor(name, shape, dtype, kind="ExternalInput")
        return dt[name]

    # data
    din("x", [N, C])
    din("idx_cn", [128, nt], I32)
    din("idx_t0", [128, ne], I32)
    din("idx_t1", [128, ne], I32)
    din("valid", [128, nt])
    din("vmbd", [128, 4 * nt])
    din("ebd", [128, 4 * nt])
    din("ind", [1, 4 * nt])
    # weights / constants
    for nm in ["w0xT", "a1", "a2", "a3", "wk_l",
               "woutA", "woutB"] + [f"wqbd_{h}" for h in range(8)]:
        din(nm, [128, 128])
    din("wv_aug", [128, 129])
    for nm in ["wff1_0", "wff1_1", "wff1_2", "wff1_3",
               "wff2_0", "wff2_1", "wff2_2", "wff2_3"]:
        din(nm, [128, 128])
    for nm in ["wx1_0", "wx1_1", "wxj1_0", "wxj1_1"]:
        din(nm, [128, 128])
    for ic in range(2):
        for oc in range(2):
            din(f"wx2_{ic}{oc}", [128, 128])
            din(f"wx3_{ic}{oc}", [128, 128])
            din(f"wxj2_{ic}{oc}", [128, 128])
            din(f"wl1_{ic}{oc}", [128, 128])
    din("wl2_0", [128, 1])
    din("wl2_1", [128, 1])
    din("sel4", [4, 128])
    din("ones1", [1, 128])
    din("tokb_row", [1, 128])
    din("outb_row", [1, 128])
    din("bff2_row", [1, 128])
    for nm in ["beta_col", "eps_col", "epsd_col"]:
        din(nm, [128, 1])
    for nm in ["bff1_0", "bff1_1", "bff1_2", "bff1_3",
               "bx1_0", "bx1_1", "bx2_0", "bx2_1", "bx3_0", "bx3_1",
               "bxj1_0", "bxj1_1", "bxj2_0", "bxj2_1", "bl1_0", "bl1_1"]:
        din(nm, [128, 1])
    din("bl2", [1, 1])

    ec_dram = nc.dram_tensor("ec_dram", [tcn, 128], F32)  # internal scratch
    xcn_dram = nc.dram_tensor("xcn_dram", [tcn, 128], F32)
    out_dram = nc.dram_tensor("out", [1, 4 * nt], F32, kind="ExternalOutput")

    with TileContext(nc) as tc:
        with (
            tc.tile_pool(name="cpool", bufs=1) as cp,
            tc.tile_pool(name="wpool", bufs=3) as wp,
            tc.tile_pool(name="mlppool", bufs=1) as mp,
            tc.tile_pool(name="carry", bufs=18) as cr,
            tc.tile_pool(name="ps", bufs=2, space="PSUM") as ps,
            tc.tile_pool(name="psbig", bufs=4, space="PSUM") as psb,
            tc.tile_pool(name="psctx", bufs=2, space="PSUM") as psc,
        ):
            # ---- load constants to SBUF --------------------------------
            cs = {}
            for nm, t in dt.items():
                if nm == "x":
                    continue
                tile = cp.tile(list(t.shape), t.dtype, tag=f"c_{nm}")
                nc.sync.dma_start(tile[:], t[:])
                cs[nm] = tile

            ident = cp.tile([128, 128], F32, tag="ident")
            make_identity(nc, ident[:])

            xijT_all = cp.tile([128, tcn], F32, tag="xijT_all")

            def transpose_to(dst_ap, src_ap, eng="act"):
                tp = ps.tile([128, 128], F32, tag="p128")
                nc.tensor.transpose(tp[:], src_ap, ident[:])
                if eng == "act":
                    nc.scalar.copy(dst_ap, tp[:])
                else:
                    nc.vector.tensor_copy(out=dst_ap, in_=tp[:])

            # ---- PHASE A: per-edge features EC + xijT ------------------
            for j in range(ne if "a" in phases else 0):
                xi = wp.tile([128, C], F32, tag="xi")
                xj = wp.tile([128, C], F32, tag="xj")
                nc.gpsimd.indirect_dma_start(
                    out=xi[:], out_offset=None, in_=dt["x"][:],
                    in_offset=bass.IndirectOffsetOnAxis(
                        ap=cs["idx_t0"][:, j:j + 1], axis=0))
                nc.gpsimd.indirect_dma_start(
                    out=xj[:], out_offset=None, in_=dt["x"][:],
                    in_offset=bass.IndirectOffsetOnAxis(
                        ap=cs["idx_t1"][:, j:j + 1], axis=0))
                xij = wp.tile([128, C], F32, tag="xij")
                nc.vector.tensor_tensor(out=xij[:], in0=xi[:], in1=xj[:],
                                        op=ALU.mult)
                xiT = wp.tile([128, 128], F32, tag="xiT")
                xjT = wp.tile([128, 128], F32, tag="xjT")
                transpose_to(xiT[:], xi[:])
                transpose_to(xjT[:], xj[:])
                transpose_to(xijT_all[:, 128 * j:128 * (j + 1)], xij[:])

                ecp = ps.tile([128, 128], F32, tag="p128")
                nc.tensor.matmul(ecp[:], lhsT=xiT[:], rhs=cs["a1"][:],
                                 start=True, stop=False)
                nc.tensor.matmul(ecp[:], lhsT=xjT[:], rhs=cs["a2"][:],
                                 start=False, stop=False)
                nc.tensor.matmul(ecp[:], lhsT=xijT_all[:, 128 * j:128 * (j + 1)],
                                 rhs=cs["a3"][:], start=False, stop=False)
                nc.tensor.matmul(ecp[:], lhsT=cs["ones1"][:],
                                 rhs=cs["tokb_row"][:], start=False, stop=True)
                ec_s = wp.tile([128, 128], F32, tag="ec_s")
                nc.scalar.copy(ec_s[:], ecp[:])
                nc.sync.dma_start(ec_dram[128 * j:128 * (j + 1), :], ec_s[:])

            # ---- PHASE B: grouped, staged over 128-token tiles ---------
            # Stages per group of G tiles so ACT table funcs (sqrt/exp/gelu)
            # batch together: 4 table loads per G tiles instead of 4/tile.
            SHUF16 = [16] * 32
            GRP = 16
            nb = nt if "b" in phases else 0

            def s1a(m):
                """gather -> tok (relu'd) + LN1 stats; returns (tok, mv)."""
                xw = wp.tile([128, C], F32, tag="xw", name="xw")
                nc.gpsimd.indirect_dma_start(
                    out=xw[:], out_offset=None, in_=dt["x"][:],
                    in_offset=bass.IndirectOffsetOnAxis(
                        ap=cs["idx_cn"][:, m:m + 1], axis=0))
                ec4 = wp.tile([4, 128], F32, tag="ec4", name="ec4")
                nc.sync.dma_start(ec4[:], ec_dram[4 * m:4 * m + 4, :])
                xwT = wp.tile([128, 128], F32, tag="xwT", name="xwT")
                transpose_to(xwT[:], xw[:], eng="dve")
                tokp = ps.tile([128, 128], F32, tag="p128", name="tokp")
                nc.tensor.matmul(tokp[:], lhsT=xwT[:], rhs=cs["w0xT"][:],
                                 start=True, stop=False)
                nc.tensor.matmul(tokp[:], lhsT=cs["sel4"][:], rhs=ec4[:],
                                 start=False, stop=True)
                tok = cr.tile([128, 128], F32, tag="tok", name="tok")
                nc.scalar.activation(tok[:], tokp[:], AF.Relu)
                st = wp.tile([128, 6], F32, tag="ln_st", name="st")
                nc.vector.bn_stats(st[:], tok[:])
                mv = cr.tile([128, 2], F32, tag="mv", name="mv")
                nc.vector.bn_aggr(mv[:], st[:])
                return tok, mv

            def sqrt_of(mv, tag):
                std = cr.tile([128, 1], F32, tag=tag, name="std")
                nc.scalar.activation(std[:], mv[:, 1:2], AF.Sqrt,
                                     bias=cs["eps_col"][:, 0:1])
                return std

            def ln_apply(x, mv, std, tag):
                rstd = wp.tile([128, 1], F32, tag="rstd_" + tag, name="rstd")
                nc.vector.reciprocal(rstd[:], std[:])
                z = wp.tile([128, 128], F32, tag="z_" + tag, name="z")
                nc.vector.tensor_scalar(out=z[:], in0=x[:],
                                        scalar1=mv[:, 0:1],
                                        scalar2=rstd[:, 0:1],
                                        op0=ALU.subtract, op1=ALU.mult)
                return z

            def s1b_1(m, tok, mv, std):
                z1 = ln_apply(tok, mv, std, "1")
                z1T = wp.tile([128, 128], F32, tag="z1T", name="z1T", bufs=5)
                transpose_to(z1T[:], z1[:])

                kp = ps.tile([128, 128], F32, tag="p128", name="kp")
                nc.tensor.matmul(kp[:], lhsT=cs["wk_l"][:], rhs=z1T[:],
                                 start=True, stop=True)
                kTs = wp.tile([128, 128], F32, tag="kTs", name="kTs", bufs=5)
                nc.scalar.copy(kTs[:], kp[:])
                qbds = wp.tile([128, 1024], F32, tag="qbds", name="qbds",
                               bufs=5)
                for half in range(2):
                    qbdp = psb.tile([128, 512], F32, tag="big", name="qbdp")
                    for hh in range(4):
                        h = 4 * half + hh
                        nc.tensor.matmul(qbdp[:, 128 * hh:128 * (hh + 1)],
                                         lhsT=cs[f"wqbd_{h}"][:], rhs=z1T[:],
                                         start=True, stop=True)
                    if half == 0:
                        nc.vector.tensor_copy(out=qbds[:, 0:512], in_=qbdp[:])
                    else:
                        nc.scalar.copy(qbds[:, 512:1024], qbdp[:])
                vp = ps.tile([128, 129], F32, tag="p128", name="vp")
                nc.tensor.matmul(vp[:], lhsT=z1T[:], rhs=cs["wv_aug"][:],
                                 start=True, stop=True)
                v_s = wp.tile([128, 129], F32, tag="v_s", name="v_s", bufs=5)
                nc.scalar.copy(v_s[:], vp[:])
                ebias = wp.tile([128, 4], F32, tag="ebias", name="ebias",
                                bufs=5)
                nc.gpsimd.tensor_tensor(
                    out=ebias[:], in0=cs["vmbd"][:, 4 * m:4 * m + 4],
                    in1=v_s[:, 128:129].to_broadcast([128, 4]), op=ALU.add)
                vaug = wp.tile([128, 8 * 17], F32, tag="vaug", name="vaug",
                               bufs=5)
                va = vaug[:].rearrange("p (h d) -> p h d", d=17)
                nc.gpsimd.tensor_copy(
                    out=va[:, :, 0:16],
                    in_=v_s[:, 0:128].rearrange("p (h d) -> p h d", d=16))
                nc.gpsimd.tensor_copy(
                    out=va[:, :, 16:17],
                    in_=cs["valid"][:, m:m + 1].to_broadcast([128, 8, 1]))
                return kTs, qbds, ebias, vaug

            def s1b_2(m, kTs, qbds, ebias):
                E = wp.tile([128, 1024], F32, tag="E", name="E", bufs=5)
                for half in range(2):
                    sp = psb.tile([128, 512], F32, tag="big", name="sp")
                    nc.tensor.matmul(sp[:], lhsT=kTs[:],
                                     rhs=qbds[:, 512 * half:512 * (half + 1)],
                                     start=True, stop=True)
                    Ev = E[:, 512 * half:512 * (half + 1)].rearrange(
                        "p (h q) -> p h q", q=128)
                    sv = sp[:].rearrange("p (h q) -> p h q", q=128)
                    for e in range(4):
                        nc.scalar.activation(
                            Ev[:, :, 32 * e:32 * (e + 1)],
                            sv[:, :, 32 * e:32 * (e + 1)],
                            AF.Exp, bias=ebias[:, e:e + 1])
                return E

            def s1b_3(m, vaug, E):
                ctxp = psc.tile([128, 256], F32, tag="ctx", name="ctxp")
                for h in range(8):
                    co = 0 if h < 4 else 128
                    hh = 32 * (h % 4)
                    nc.tensor.matmul(
                        ctxp[hh:hh + 17, co:co + 128],
                        lhsT=vaug[:, 17 * h:17 * (h + 1)],
                        rhs=E[:, 128 * h:128 * (h + 1)],
                        start=True, stop=True, tile_position=(0, hh))
                cx = wp.tile([128, 256], F32, tag="cx", name="cx", bufs=5)
                nc.scalar.activation(cx[:], ctxp[:], AF.Identity,
                                     bias=cs["epsd_col"][:, 0:1])
                rt = wp.tile([128, 256], F32, tag="rt", name="rt")
                nc.vector.stream_shuffle(rt[:], cx[:], SHUF16)
                rtr = wp.tile([128, 256], F32, tag="rtr", name="rtr")
                nc.vector.reciprocal(rtr[:], rt[:])
                cn = wp.tile([128, 256], F32, tag="cn", name="cn", bufs=5)
                nc.vector.tensor_tensor(out=cn[:], in0=cx[:], in1=rtr[:],
                                        op=ALU.mult)
                return cn

            def s1b_4(m, tok, cn):
                up = ps.tile([128, 128], F32, tag="p128", name="up")
                nc.tensor.matmul(up[:], lhsT=cn[:, 0:128], rhs=cs["woutA"][:],
                                 start=True, stop=False)
                nc.tensor.matmul(up[:], lhsT=cn[:, 128:256],
                                 rhs=cs["woutB"][:], start=False, stop=False)
                nc.tensor.matmul(up[:], lhsT=cs["ones1"][:],
                                 rhs=cs["outb_row"][:], start=False, stop=True)
                tok2 = cr.tile([128, 128], F32, tag="tok2", name="tok2")
                nc.vector.tensor_tensor(out=tok2[:], in0=tok[:], in1=up[:],
                                        op=ALU.add)
                st2 = wp.tile([128, 6], F32, tag="ln_st2", name="st2")
                nc.vector.bn_stats(st2[:], tok2[:])
                mv2 = cr.tile([128, 2], F32, tag="mv2", name="mv2")
                nc.vector.bn_aggr(mv2[:], st2[:])
                return tok2, mv2

            def s1b_group(ms, d1, stds):
                SG = 4
                out = {}
                for i0 in range(0, len(ms), SG):
                    sub = ms[i0:i0 + SG]
                    st1 = {m: s1b_1(m, d1[m][0], d1[m][1], stds[m])
                           for m in sub}
                    eE = {m: s1b_2(m, st1[m][0], st1[m][1], st1[m][2])
                          for m in sub}
                    cns = {m: s1b_3(m, st1[m][3], eE[m]) for m in sub}
                    for m in sub:
                        out[m] = s1b_4(m, d1[m][0], cns[m])
                return out

            def s2(m, tok2, mv2, std2):
                """LN2 apply + ff + residual + pool."""
                z2 = ln_apply(tok2, mv2, std2, "2")
                z2T = wp.tile([128, 128], F32, tag="z2T", name="z2T")
                transpose_to(z2T[:], z2[:])
                gT = wp.tile([128, 512], F32, tag="gT", name="gT")
                fp = psb.tile([128, 512], F32, tag="big", name="fp")
                for c4 in range(4):
                    nc.tensor.matmul(fp[:, 128 * c4:128 * (c4 + 1)],
                                     lhsT=cs[f"wff1_{c4}"][:],
                                     rhs=z2T[:], start=True, stop=True)
                    nc.scalar.activation(gT[:, 128 * c4:128 * (c4 + 1)],
                                         fp[:, 128 * c4:128 * (c4 + 1)],
                                         AF.Gelu,
                                         bias=cs[f"bff1_{c4}"][:, 0:1])
                f2p = ps.tile([128, 128], F32, tag="p128", name="f2p")
                for c4 in range(4):
                    nc.tensor.matmul(f2p[:],
                                     lhsT=gT[:, 128 * c4:128 * (c4 + 1)],
                                     rhs=cs[f"wff2_{c4}"][:],
                                     start=(c4 == 0), stop=(c4 == 3))
                tok3 = wp.tile([128, 128], F32, tag="tok3", name="tok3")
                nc.vector.tensor_tensor(out=tok3[:], in0=tok2[:], in1=f2p[:],
                                        op=ALU.add)
                pp = ps.tile([4, 128], F32, tag="p128", name="pp")
                nc.tensor.matmul(pp[:], lhsT=cs["ebd"][:, 4 * m:4 * m + 4],
                                 rhs=tok3[:], start=True, stop=False)
                nc.tensor.matmul(pp[:], lhsT=cs["ind"][0:1, 4 * m:4 * m + 4],
                                 rhs=cs["bff2_row"][:], start=False, stop=True)
                pxs = wp.tile([4, 128], F32, tag="pxs", name="pxs")
                nc.scalar.copy(pxs[:], pp[:])
                nc.sync.dma_start(xcn_dram[4 * m:4 * m + 4, :], pxs[:])

            for g0 in range(0, nb, GRP):
                gms = list(range(g0, min(g0 + GRP, nb)))
                d1 = {m: s1a(m) for m in gms}
                stds = {m: sqrt_of(d1[m][1], "std1") for m in gms}
                d2 = s1b_group(gms, d1, stds)
                stds2 = {m: sqrt_of(d2[m][1], "std2") for m in gms}
                for m in gms:
                    s2(m, d2[m][0], d2[m][1], stds2[m])

            # ---- PHASE C ------------------------------------
            def _phase_c(lo, w):
                # ---- PHASE C: edge MLPs (edges [lo, lo+w)) -----------------
                xcnT = mp.tile([128, w], F32, tag="xcnT", name="xcnT")
                for j in range(lo // 128, (lo + w) // 128):
                    xct = wp.tile([128, 128], F32, tag="xct", name="xct")
                    nc.sync.dma_start(xct[:], xcn_dram[128 * j:128 * (j + 1), :])
                    transpose_to(xcnT[:, 128 * j - lo:128 * (j + 1) - lo],
                                 xct[:])

                def dense(rhs_tile, win, bin_, act, n_ic, out_tag):
                    """out[oc-chunk][128, w] = act(W @ rhs + b)."""
                    outs = []
                    for oc in range(2):
                        o = mp.tile([128, w], F32, tag=f"{out_tag}{oc}",
                                    name=out_tag)
                        for nh in range(max(1, w // 512)):
                            cw = min(512, w)
                            p5 = psb.tile([128, 512], F32, tag="big")
                            for ic in range(n_ic):
                                wt = cs[win(ic, oc)]
                                r = (rhs_tile if n_ic == 1 else rhs_tile[ic])
                                nc.tensor.matmul(
                                    p5[:, :cw], lhsT=wt[:],
                                    rhs=r[:, cw * nh:cw * (nh + 1)],
                                    start=(ic == 0), stop=(ic == n_ic - 1))
                            nc.scalar.activation(
                                o[:, cw * nh:cw * (nh + 1)], p5[:, :cw], act,
                                bias=cs[bin_(oc)][:, 0:1])
                        outs.append(o)
                    return outs

                h1 = dense(xcnT, lambda ic, oc: f"wx1_{oc}",
                           lambda oc: f"bx1_{oc}", AF.Relu, 1, "h1_")
                h2 = dense(h1, lambda ic, oc: f"wx2_{ic}{oc}",
                           lambda oc: f"bx2_{oc}", AF.Relu, 2, "h2_")
                h3 = dense(h2, lambda ic, oc: f"wx3_{ic}{oc}",
                           lambda oc: f"bx3_{oc}", AF.Identity, 2, "h3_")
                j1 = dense(xijT_all[:, lo:lo + w], lambda ic, oc: f"wxj1_{oc}",
                           lambda oc: f"bxj1_{oc}", AF.Relu, 1, "j1_")
                j2 = dense(j1, lambda ic, oc: f"wxj2_{ic}{oc}",
                           lambda oc: f"bxj2_{oc}", AF.Identity, 2, "j2_")
                zi = []
                for oc in range(2):
                    z = mp.tile([128, w], F32, tag=f"zi{oc}", name="zi")
                    nc.vector.scalar_tensor_tensor(
                        out=z[:], in0=h3[oc][:], scalar=cs["beta_col"][:, 0:1],
                        in1=j2[oc][:], op0=ALU.mult, op1=ALU.add)
                    zi.append(z)
                zz = dense(zi, lambda ic, oc: f"wl1_{ic}{oc}",
                           lambda oc: f"bl1_{oc}", AF.Relu, 2, "zz")

                osb = mp.tile([1, w], F32, tag="osb", name="osb")
                cw = min(512, w)
                for nh in range(max(1, w // 512)):
                    fo = ps.tile([1, 512], F32, tag="p128")
                    nc.tensor.matmul(fo[:, :cw], lhsT=cs["wl2_0"][:],
                                     rhs=zz[0][:, cw * nh:cw * (nh + 1)],
                                     start=True, stop=False)
                    nc.tensor.matmul(fo[:, :cw], lhsT=cs["wl2_1"][:],
                                     rhs=zz[1][:, cw * nh:cw * (nh + 1)],
                                     start=False, stop=True)
                    nc.scalar.activation(osb[0:1, cw * nh:cw * (nh + 1)],
                                         fo[:, :cw],
                                         AF.Identity, bias=cs["bl2"][0:1, 0:1])
                nc.sync.dma_start(out_dram[0:1, lo:lo + w], osb[:])

            if "c" in phases:
                for _lo in range(0, tcn, 512):
                    _phase_c(_lo, min(512, tcn - _lo))
            else:
                dumm = mp.tile([1, tcn], F32, tag="dumm")
                nc.vector.memset(dumm[:], 0.0)
                nc.sync.dma_start(out_dram[:], dumm[:])

    nc.finalize()
    return nc


def _ln_stats(nc, wp, x, z_out, eps_col):
    """z = (x - mean(x)) / sqrt(var(x) + 1e-5) along the free dim."""
    st = wp.tile([128, 6], F32, tag="ln_st")
    nc.vector.bn_stats(st[:], x[:])
    mv = wp.tile([128, 2], F32, tag="ln_mv")
    nc.vector.bn_aggr(mv[:], st[:])
    std = wp.tile([128, 1], F32, tag="ln_std")
    nc.scalar.activation(std[:], mv[:, 1:2], AF.Sqrt, bias=eps_col[:, 0:1])
    rstd = wp.tile([128, 1], F32, tag="ln_rstd")
    nc.vector.reciprocal(rstd[:], std[:])
    nc.vector.tensor_scalar(out=z_out[:], in0=x[:], scalar1=mv[:, 0:1],
                            scalar2=rstd[:, 0:1], op0=ALU.subtract,
                            op1=ALU.mult)


# ---------------------------------------------------------------- host side

def _slot_pad_w(Weff, beff, heads):
    """[128c, 128slot] lhsT with 4 heads in 32-slots (16 data + 16 zero)."""
    w = np.zeros((128, 128), np.float32)
    b = np.zeros((128, 1), np.float32)
    for i, h in enumerate(heads):
        w[:, 32 * i:32 * i + 16] = Weff[16 * h:16 * h + 16, :].T
        b[32 * i:32 * i + 16, 0] = beff[16 * h:16 * h + 16]
    return w, b


def _prep_shared(inp):
    f = lambda k: np.asarray(inp[k], np.float32)
    tok_w, tok_b = f("tok_w"), f("tok_b")
    g1, b1 = f("ln1_g"), f("ln1_b")
    qkv_w, qkv_b = f("qkv_w"), f("qkv_b")
    out_w, out_b = f("out_w"), f("out_b")
    g2, b2 = f("ln2_g"), f("ln2_b")
    ff1_w, ff1_b = f("ff1_w"), f("ff1_b")
    ff2_w, ff2_b = f("ff2_w"), f("ff2_b")

    d = {}
    d["w0xT"] = tok_w[:, :C].T.copy()
    d["a1"] = tok_w[:, C:2 * C].T.copy()
    d["a2"] = tok_w[:, 2 * C:3 * C].T.copy()
    d["a3"] = tok_w[:, 3 * C:4 * C].T.copy()
    d["tokb_row"] = tok_b[None, :].copy()

    sc = 1.0 / np.sqrt(DH)
    Wq, Wk, Wv = qkv_w[:C], qkv_w[C:2 * C], qkv_w[2 * C:3 * C]
    bq, bk, bv = qkv_b[:C], qkv_b[C:2 * C], qkv_b[2 * C:3 * C]
    Wq_e = Wq * g1[None, :] * sc
    bq_e = (Wq @ b1) * sc + bq * sc
    Wk_e = Wk * g1[None, :]
    Wv_e = Wv * g1[None, :]
    bv_e = Wv @ b1 + bv
    d["wk_l"] = Wk_e.T.copy()
    for h in range(8):
        w = np.zeros((128, 128), np.float32)
        rows = slice(16 * h, 16 * (h + 1))
        w[:, rows] = Wq_e[rows, :].T
        d[f"wqbd_{h}"] = w
    # q/k additive biases: per-qtok terms cancel in softmax; the per-ktok
    # term c_k = bq_e . k(token) is linear in z1 -> extra v output channel.
    w_ck = Wk_e.T @ bq_e                      # [128 in-c]
    d["wv_aug"] = np.concatenate([Wv_e.T, w_ck[:, None]], axis=1).copy()

    for nm, heads in (("woutA", [0, 1, 2, 3]), ("woutB", [4, 5, 6, 7])):
        w = np.zeros((128, 128), np.float32)
        for i, h in enumerate(heads):
            w[32 * i:32 * i + 16, :] = out_w[:, 16 * h:16 * h + 16].T
        d[nm] = w
    d["outb_row"] = (out_b + out_w @ bv_e)[None, :].copy()

    for c4 in range(4):
        sl = slice(128 * c4, 128 * (c4 + 1))
        d[f"wff1_{c4}"] = (ff1_w[sl, :] * g2[None, :]).T.copy()
        d[f"bff1_{c4}"] = (ff1_w[sl, :] @ b2 + ff1_b[sl])[:, None].copy()
        d[f"wff2_{c4}"] = ff2_w[:, sl].T.copy()
    d["bff2_row"] = ff2_b[None, :].copy()

    for nm, wkey, bkey in (("wx1", "xcn_w1", "xcn_b1"),
                           ("wxj1", "xij_w1", "xij_b1")):
        W, B = f(wkey), f(bkey)
        for oc in range(2):
            sl = slice(128 * oc, 128 * (oc + 1))
            d[f"{nm}_{oc}"] = W[sl, :].T.copy()
            d[f"b{nm[1:]}_{oc}"] = B[sl][:, None].copy()
    for nm, wkey, bkey in (("wx2", "xcn_w2", "xcn_b2"),
                           ("wx3", "xcn_w3", "xcn_b3"),
                           ("wxj2", "xij_w2", "xij_b2"),
                           ("wl1", "lin_w1", "lin_b1")):
        W, B = f(wkey), f(bkey)
        for ic in range(2):
            for oc in range(2):
                d[f"{nm}_{ic}{oc}"] = \
                    W[128 * oc:128 * (oc + 1), 128 * ic:128 * (ic + 1)].T.copy()
        for oc in range(2):
            d[f"b{nm[1:]}_{oc}"] = B[128 * oc:128 * (oc + 1)][:, None].copy()
    lin_w2, lin_b2 = f("lin_w2"), f("lin_b2")
    d["wl2_0"] = lin_w2[0, :128][:, None].copy()
    d["wl2_1"] = lin_w2[0, 128:][:, None].copy()
    d["bl2"] = lin_b2.reshape(1, 1).copy()

    sel4 = np.zeros((4, 128), np.float32)
    for e in range(4):
        sel4[e, 32 * e:32 * (e + 1)] = 1.0
    d["sel4"] = sel4
    d["ones1"] = np.ones((1, 128), np.float32)
    d["eps_col"] = np.full((128, 1), 1e-5, np.float32)
    d["epsd_col"] = np.full((128, 1), 1e-30, np.float32)
    d["beta_col"] = np.full((128, 1), np.asarray(inp["beta"],
                                                 np.float32).reshape(-1)[0])
    return {k: np.ascontiguousarray(v, np.float32) for k, v in d.items()}


def _prep_core(inp, core, nt=NT):
    ne = 4 * nt // 128
    sl = slice(core * TC, (core + 1) * TC)
    tar = np.asarray(inp["tar_ei"])[:, sl].astype(np.int32)
    cols = np.asarray(inp["cn_cols"])[sl].astype(np.int32)     # [TC, K]
    cnt = np.asarray(inp["cn_counts"])[sl].astype(np.int64)    # [TC]

    d = {}
    d["idx_cn"] = np.ascontiguousarray(cols.reshape(-1).reshape(NT, 128).T)[:, :nt]
    d["idx_t0"] = np.ascontiguousarray(tar[0].reshape(NE, 128).T)[:, :ne]
    d["idx_t1"] = np.ascontiguousarray(tar[1].reshape(NE, 128).T)[:, :ne]

    kk = np.arange(K)
    valid_ek = (kk[None, :] < cnt[:, None])                    # [TC, K] bool
    valid_flat = valid_ek.reshape(-1)                          # token-order
    d["valid"] = np.ascontiguousarray(
        valid_flat.reshape(NT, 128).T.astype(np.float32))

    p = np.arange(128)
    pe = p // 32                                               # edge slot of row
    vmbd = np.zeros((128, 4 * NT), np.float32)
    ebd = np.zeros((128, 4 * NT), np.float32)
    vf = d["valid"]                                            # [128, NT]
    rc = (1.0 / np.maximum(cnt, 1)).astype(np.float32)         # [TC]
    for e in range(4):
        onblk = (pe == e)                                      # [128]
        vmbd[:, e::4] = NEG * (~(onblk[:, None] & (vf > 0))).astype(np.float32)
        ebd[:, e::4] = (onblk[:, None] * vf) * rc.reshape(NT, 4).T[e][None, :]
    d["vmbd"] = vmbd[:, :4 * nt]
    d["ebd"] = ebd[:, :4 * nt]
    d["ind"] = (cnt > 0).astype(np.float32)[None, :4 * nt].copy()
    d["valid"] = d["valid"][:, :nt].copy()
    return {k: np.ascontiguousarray(v) for k, v in d.items()}


_CACHE = {}
_CACHE_LOCK = threading.Lock()


def _get_nc(nt=NT, phases="abc", bsteps=99):
    with _CACHE_LOCK:
        key = (nt, phases, bsteps)
        if key not in _CACHE:
            _CACHE[key] = _build_nc(nt, phases, bsteps)
        return _CACHE[key]


def run(inputs, nt=NT, phases="abc", bsteps=99, **spmd_kwargs):
    """Run the kernel on the first 4*nt edges of each core's shard.

    Returns (out [NCORES, 4*nt], BassKernelResults).
    """
    nc = _get_nc(nt, phases, bsteps)
    shared = _prep_shared(inputs)
    x = np.ascontiguousarray(np.asarray(inputs["x"], np.float32))
    in_maps = []
    for core in range(NCORES):
        m = dict(shared)
        m["x"] = x
        m.update(_prep_core(inputs, core, nt))
        in_maps.append(m)
    res = run_bass_kernel_spmd(nc, in_maps, core_ids=list(range(NCORES)),
                               **spmd_kwargs)
    out = np.stack([res.results[c]["out"][0] for c in range(NCORES)])
    return out, res


def kernel(**inputs):
    out, _ = run(inputs)
    return out.reshape(T, O).astype(np.float32)

